# revision 9
# baseline (speedup 1.0000x reference)
# Trainium2 Bass kernel for EpiFeatureRebuild (two chained EPI-query stages).
#
# Sharding: core k owns w-columns [8k, 8k+8). Stage-1 computes queries
# p in that window for all 320 horizontal EPIs; stage-2 EPIs (a1, p1)
# with p1 in the window are then fully local -> no cross-core traffic.
#
# Per-EPI pipeline (identical math both stages):
#   Z = conv3x3(feat) @ W0 (shift-decomposed, 6 matmuls: channels of
#       rows (di=0, di=1) stacked on 128 partitions + K=64 for di=2)
#   H1[:, (a,w)] = relu(Z[:, (iy_a, w)] + R[:, a])   (R folds b0 + rel0*W0[576])
#   H_{l+1} = relu(W_l^T H_l + b_l)  l=1..3, out = W4^T H4 + b4
# Matmuls run in float32r (FP22 read) at full PE rate (N>=256).
import numpy as np

C = 64
A = 9
U = 5          # angular size (v or u) of an EPI
HW = 64        # spatial size (w or h)
NCORES = 8
WLOC = 8       # w-columns per core
IY = [0, 0, 1, 1, 2, 3, 3, 4, 4]

# Stage-1 geometry: per-EPI padded block = 1 + 7*10 + 1 = 72 cols.
# row r (0..6) at 1+10r, interior v=r-1, w-halo cols c=0..9.
BW1 = 72
NB1 = 20       # stage-1 batches (u, hgroup): 5 u * 4 groups of 16 h
E1 = 16        # EPIs per stage-1 batch
NPOS1 = 50     # conv output window per EPI (flat 11..60), col j = 10v + c

# Stage-2 geometry: per-block padded = 1 + 7*66 + 1 = 464 cols.
BW2 = 464
NB2 = 36       # stage-2 groups
E2 = 2         # blocks per group
NPOS2 = 330    # conv window (flat 67..396), col j = 66u + c

WPCOLS = 3232

_CACHE = {}


def _rel0():
    a = np.arange(A, dtype=np.float32)
    c0 = np.float32(-1.0 + 1.0 / A) + np.float32(2.0 / A) * a
    iy = np.array(IY, np.float32)
    qc0 = (np.float32(2.0) * iy + np.float32(1.0)) / np.float32(U) - np.float32(1.0)
    return (c0 - qc0) * np.float32(U)


def _build_nc():
    import concourse.bass as bass
    import concourse.tile as tile
    from concourse import bacc, mybir

    f32 = mybir.dt.float32
    f32r = mybir.dt.float32r

    def r(ap):
        return ap

    nc = bacc.Bacc("TRN2", target_bir_lowering=False, debug=False)
    xs_d = nc.declare_dram_parameter("xs", [NB1, 128, E1, BW1], f32r, isOutput=False)
    wp_d = nc.declare_dram_parameter("wp", [128, WPCOLS], f32r, isOutput=False)
    out_d = nc.declare_dram_parameter("out", [64, A * WLOC, A, HW], f32, isOutput=True)

    Relu = mybir.ActivationFunctionType.Relu
    Ident = mybir.ActivationFunctionType.Identity
    add_op = mybir.AluOpType.add
    max_op = mybir.AluOpType.max

    eng_ctr = [0]

    def epilogue(out_ap, in_ap, bias_ap, relu):
        # alternate ACT / DVE for load balance
        eng_ctr[0] += 1
        if eng_ctr[0] % 2 == 0:
            if relu:
                nc.scalar.activation(out_ap, in_ap, Relu, bias=bias_ap)
            else:
                nc.scalar.activation(out_ap, in_ap, Ident, bias=bias_ap)
        else:
            if relu:
                nc.vector.tensor_scalar(out_ap, in_ap, bias_ap, 0.0, add_op, max_op)
            else:
                nc.vector.tensor_scalar(out_ap, in_ap, bias_ap, None, add_op)

    with tile.TileContext(nc) as tc:
        with (
            tc.tile_pool(name="const", bufs=1) as cpool,
            tc.tile_pool(name="res", bufs=1) as rpool,
            tc.tile_pool(name="xin", bufs=2) as xpool,
            tc.tile_pool(name="hbuf", bufs=6) as hpool,
            tc.tile_pool(name="f2", bufs=1) as f2pool,
            tc.tile_pool(name="ostg", bufs=2) as opool,
            tc.tile_pool(name="zps", bufs=2, space="PSUM") as zpool,
            tc.tile_pool(name="hps", bufs=3, space="PSUM") as hps,
            tc.tile_pool(name="l4ps", bufs=1, space="PSUM") as l4ps,
        ):
            # ---- constants: one packed DMA ----
            wp = cpool.tile([128, WPCOLS], f32r)
            nc.sync.dma_start(wp[:], wp_d[:])
            wa = wp[:, 0:768].rearrange("p (d m) -> p d m", d=3)
            wb = wp[:, 768:1536].rearrange("p (d m) -> p d m", d=3)
            wl = [wp[:, 1536 + 512 * i:1536 + 512 * (i + 1)].rearrange(
                "p (k m) -> p k m", k=2) for i in range(3)]
            w4 = wp[:, 3072:3200].rearrange("p (k m) -> p k m", k=2)
            rt = wp[:, 3200:3218].rearrange("p (k a) -> p k a", k=2).bitcast(f32)
            bl = wp[:, 3218:3224].rearrange("p (k l) -> p k l", k=2).bitcast(f32)
            b4 = wp[0:64, 3224:3225].bitcast(f32)

            # ---- stage-1 output, resident: RES[o(64), block(a,w) 72, (u,h) 320]
            res = rpool.tile([64, A * WLOC, U * HW], f32r)

            # ---- stage-2 conv input slots (pads must stay zero across reuse)
            f2t = [f2pool.tile([128, E2, BW2], f32r, name=f"f2_{i}", tag=f"f2_{i}") for i in range(2)]
            nc.vector.memset(f2t[0][:].bitcast(f32), 0.0)
            nc.vector.memset(f2t[1][:].bitcast(f32), 0.0)

            def hidden_layers(h1, ncols, nchunk, csz):
                # h1: [128, 2, ncols]; returns h4 tile of same shape
                hprev = h1
                for li in range(3):
                    hn = hpool.tile([128, 2, ncols], f32r, tag="h")
                    for cc in range(nchunk):
                        for mt in range(2):
                            ps = hps.tile([128, csz], f32, tag="hp")
                            for k in range(2):
                                nc.tensor.matmul(
                                    ps[:],
                                    r(wl[li][:, k, mt * 128:(mt + 1) * 128]),
                                    r(hprev[:, k, cc * csz:(cc + 1) * csz]),
                                    start=(k == 0), stop=(k == 1))
                            epilogue(hn[:, mt, cc * csz:(cc + 1) * csz], ps[:],
                                     bl[:, mt, li:li + 1], True)
                    hprev = hn
                return hprev

            # ================= STAGE 1 =================
            for b in range(NB1):
                u, hg = b // 4, b % 4
                xin = xpool.tile([128, E1, BW1], f32r, tag="xin")
                nc.sync.dma_start(xin[:], xs_d[b])

                h1 = hpool.tile([128, 2, E1 * A * WLOC], f32r, tag="h")
                for mt in range(2):
                    zt = zpool.tile([128, 2, 512], f32, tag="z")
                    for bank in range(2):
                        es = bank * 8
                        zv = zt[:, bank, 0:8 * NPOS1]
                        for dj in range(3):
                            nc.tensor.matmul(
                                zv, r(wa[:, dj, mt * 128:(mt + 1) * 128]),
                                r(xin[:, es:es + 8, dj:dj + NPOS1]),
                                start=(dj == 0), stop=False)
                        for dj in range(3):
                            nc.tensor.matmul(
                                zv, r(wb[64:128, dj, mt * 128:(mt + 1) * 128]),
                                r(xin[64:128, es:es + 8, 10 + dj:10 + dj + NPOS1]),
                                start=False, stop=(dj == 2))
                    # gather + bias + relu -> H1 cols (e, a, w)
                    zg = zt[:, :, 0:8 * NPOS1].rearrange(
                        "p b (e v c) -> p b e v c", e=8, v=U)
                    h1v = h1[:, mt, :].rearrange(
                        "p (bk e a w) -> p bk e a w", bk=2, e=8, a=A)
                    for a in range(A):
                        epilogue(h1v[:, :, :, a, :],
                                 zg[:, :, :, IY[a], 1:1 + WLOC],
                                 rt[:, mt, a:a + 1], True)

                h4 = hidden_layers(h1, E1 * A * WLOC, 3, 384)

                # L4 -> scatter into RES  (chunk = 4 EPIs = 288 cols)
                for cc in range(4):
                    ps = l4ps.tile([64, 288], f32, tag="l4")
                    for k in range(2):
                        nc.tensor.matmul(
                            ps[:], r(w4[:, k, :]),
                            r(h4[:, k, cc * 288:(cc + 1) * 288]),
                            start=(k == 0), stop=(k == 1))
                    base = u * HW + hg * E1 + cc * 4
                    dst = res[:, :, base:base + 4].rearrange(
                        "p (a w) e -> p a w e", a=A).transpose((0, 3, 1, 2))
                    psv = ps[:].rearrange("p (e a w) -> p e a w", e=4, a=A)
                    epilogue(dst, psv, b4[:, 0:1], False)

            # ================= STAGE 2 =================
            for g in range(NB2):
                f2 = f2t[g % 2]
                # build padded conv input from RES (top rows + shifted copy)
                for blk in range(E2):
                    srcv = res[:, 2 * g + blk, :].rearrange(
                        "p (u h) -> p u h", u=U)
                    dst_top = f2[0:64, blk, 1:463].rearrange(
                        "p (u h) -> p u h", u=7, h=66)[:, 1:6, 1:65]
                    # dest col for (u,h): 68+66u+h = 1 + (row u+1)*66 + (1+h)
                    nc.sync.dma_start(dst_top, srcv)
                    dst_bot = f2[64:128, blk, 1:463].rearrange(
                        "p (u h) -> p u h", u=7, h=66)[:, 0:5, 1:65]
                    nc.sync.dma_start(dst_bot, srcv)

                h1 = hpool.tile([128, 2, E2 * A * HW], f32r, tag="h")
                for mt in range(2):
                    zt = zpool.tile([128, 2, 512], f32, tag="z")
                    for blk in range(2):
                        zv = zt[:, blk, 0:NPOS2]
                        for dj in range(3):
                            nc.tensor.matmul(
                                zv, r(wa[:, dj, mt * 128:(mt + 1) * 128]),
                                r(f2[:, blk, dj:dj + NPOS2]),
                                start=(dj == 0), stop=False)
                        for dj in range(3):
                            nc.tensor.matmul(
                                zv, r(wb[64:128, dj, mt * 128:(mt + 1) * 128]),
                                r(f2[64:128, blk, 66 + dj:66 + dj + NPOS2]),
                                start=False, stop=(dj == 2))
                    h1v = h1[:, mt, :].rearrange(
                        "p (bk a h) -> p bk a h", bk=E2, a=A)
                    for a in range(A):
                        epilogue(h1v[:, :, a, :],
                                 zt[:, :, 1 + 66 * IY[a]:1 + 66 * IY[a] + HW],
                                 rt[:, mt, a:a + 1], True)

                h4 = hidden_layers(h1, E2 * A * HW, 3, 384)

                stg = opool.tile([64, E2 * A * HW], f32, tag="ostg")
                for cc in range(4):
                    ps = l4ps.tile([64, 288], f32, tag="l4")
                    for k in range(2):
                        nc.tensor.matmul(
                            ps[:], r(w4[:, k, :]),
                            r(h4[:, k, cc * 288:(cc + 1) * 288]),
                            start=(k == 0), stop=(k == 1))
                    epilogue(stg[:, cc * 288:(cc + 1) * 288], ps[:], b4[:, 0:1], False)
                nc.sync.dma_start(
                    out_d[:, 2 * g:2 * g + 2, :, :],
                    stg[:].rearrange("p (b a h) -> p b a h", b=E2, a=A))
    nc.compile()
    return nc


def get_nc():
    if "nc" not in _CACHE:
        _CACHE["nc"] = _build_nc()
    return _CACHE["nc"]


def host_prep(x, ws, bs):
    """Returns in_maps: list of 8 dicts of numpy arrays."""
    f = np.float32
    W0 = np.asarray(ws[0], f)
    W0r = np.ascontiguousarray(W0[:576].reshape(C, 3, 3, 256))   # [c, di, dj, m]
    wa = np.zeros((128, 3, 256), f)
    wa[:64] = W0r[:, 0]
    wa[64:] = W0r[:, 1]
    wb = np.zeros((128, 3, 256), f)
    wb[64:] = W0r[:, 2]
    rel0 = _rel0()
    R = np.asarray(bs[0], f)[None, :] + rel0[:, None] * W0[576]   # [9, 256]
    rt = np.ascontiguousarray(R.T.reshape(2, 128, A).transpose(1, 0, 2))
    wlt = [np.ascontiguousarray(np.asarray(ws[l], f).reshape(2, 128, 256)
                                .transpose(1, 0, 2)) for l in (1, 2, 3)]
    w4t = np.ascontiguousarray(np.asarray(ws[4], f).reshape(2, 128, 64)
                               .transpose(1, 0, 2))
    blt = np.ascontiguousarray(
        np.stack([np.asarray(bs[l], f).reshape(2, 128) for l in (1, 2, 3)],
                 axis=-1).transpose(1, 0, 2))                     # [128, 2, 3]
    b4t = np.ascontiguousarray(np.asarray(bs[4], f).reshape(64, 1))
    wp = np.zeros((128, WPCOLS), f)
    wp[:, 0:768] = wa.reshape(128, 768)
    wp[:, 768:1536] = wb.reshape(128, 768)
    for i in range(3):
        wp[:, 1536 + 512 * i:1536 + 512 * (i + 1)] = wlt[i].reshape(128, 512)
    wp[:, 3072:3200] = w4t.reshape(128, 128)
    wp[:, 3200:3218] = rt.reshape(128, 18)
    wp[:, 3218:3224] = blt.reshape(128, 6)
    wp[0:64, 3224] = b4t[:, 0]

    x0 = np.asarray(x, f)[0]                                      # [C, U, V, H, W]
    xp = np.zeros((C, U, U, HW, HW + 2), f)
    xp[..., 1:HW + 1] = x0
    in_maps = []
    for k in range(NCORES):
        win = xp[..., 8 * k:8 * k + 10]                           # [C,U,V,H,10]
        top = np.zeros((C, U, HW, 7, 10), f)
        top[:, :, :, 1:6, :] = win.transpose(0, 1, 3, 2, 4)       # [C,U,H,V,10]
        blk = np.zeros((C, U, HW, BW1), f)
        blk[..., 1:71] = top.reshape(C, U, HW, 70)
        bot = np.zeros_like(blk)
        bot[..., 0:62] = blk[..., 10:72]
        full = np.concatenate([blk, bot], axis=0)                 # [128,U,H,72]
        xs = np.ascontiguousarray(
            full.transpose(1, 2, 0, 3).reshape(U, 4, E1, 128, BW1)
            .transpose(0, 1, 3, 2, 4).reshape(NB1, 128, E1, BW1))
        in_maps.append({"xs": xs, "wp": wp})
    return in_maps


def assemble(results):
    out = np.empty((1, C, A, A, HW, HW), np.float32)
    for k in range(NCORES):
        ok = np.asarray(results[k]["out"]).reshape(C, A, WLOC, A, HW)
        out[0, :, :, :, :, 8 * k:8 * k + 8] = ok.transpose(0, 3, 1, 4, 2)
    return out


def run(in_maps, trace=False, **kw):
    from concourse.bass_utils import run_bass_kernel_spmd
    return run_bass_kernel_spmd(get_nc(), in_maps, list(range(NCORES)),
                                trace=trace, **kw)


def kernel(x, w0, b0, w1, b1, w2, b2, w3, b3, w4, b4, patchsize=64, ang_factor=9):
    ws = [w0, w1, w2, w3, w4]
    bs = [b0, b1, b2, b3, b4]
    in_maps = host_prep(x, ws, bs)
    res = run(in_maps)
    return assemble(res.results)
